# revision 2
# baseline (speedup 1.0000x reference)
"""DiagonalS6SSM kernel entry point (dev version; imports dssm builder)."""
import os
import numpy as np

from dssm import Cfg, prep_core_inputs, required_b_tile, build_kernel, make_bias_on

LAST_RUN = {}
_CACHE = {}


def kernel(**inputs) -> np.ndarray:
    from concourse.bass_utils import run_bass_kernel_spmd

    inp = {k: np.asarray(v) for k, v in inputs.items()}
    T, V, CIN = inp['xs'].shape
    E = inp['edge_index'].shape[2]
    NCORES, NTILES, VTILE = 8, 10, 125

    bt = required_b_tile(inp['edge_index'], V, NCORES, VTILE, NTILES, T)
    bias_on = make_bias_on(inp)
    cfg = Cfg(V=V, E=E, B_TILE=bt, NTILES=NTILES, VTILE=VTILE, NG=5, HCH=8,
              T=T, ssm_bf16=True)

    key = (bt, tuple(sorted(bias_on.items())))
    if key not in _CACHE:
        _CACHE[key] = build_kernel(cfg, bias_on)
    nc = _CACHE[key]

    maps = [prep_core_inputs(cfg, inp, k) for k in range(NCORES)]
    trace = bool(os.environ.get("DSSM_TRACE"))
    if trace:
        import axon_prof
        axon_prof.install()
    res = run_bass_kernel_spmd(nc, maps, list(range(NCORES)), trace=trace)
    LAST_RUN['exec_time_ns'] = res.exec_time_ns
    LAST_RUN['profile_json'] = res.profile_json
    out = np.concatenate([res.results[k]['out'] for k in range(NCORES)], 0)
    return out.astype(np.float32)
